# revision 12
# baseline (speedup 1.0000x reference)
"""ContxE-style temporal KG embedding scoring kernel for Trainium2 (Bass/Tile).

Contract: kernel(**inputs) takes FULL unsharded numpy inputs and returns the
FULL [B] float32 output. Internally shards the batch across 8 NeuronCores
(data-parallel, tables replicated) and runs a Bass/Tile kernel via
run_bass_kernel_spmd.

Math (per batch element b, window W=5, D=512):
  idx[b,w] = d[b]-(4-w), clamped: negatives -> 365
  c/s[b,w,:] = cos/sin(time_table[idx[b,w]])
  h_real = hr*c - hi*s ; h_img = hr*s + hi*c   (same for t)
  4 attention softmaxes over W of <r, rotated>, then weighted sums,
  out = sum|y_r + rr - z_r| + sum|y_i + ri + z_i|

Device-side strategy (per core, B_loc=2048 = 16 tiles of 128):
  - ONE indirect-DMA gather per embedding pair (tables concatenated host-side)
  - cos|sin rows come from a host-precomputed bf16 table with 4 prefix rows
    equal to row 365, so the W-window gather is ONE contiguous 10KB read per
    batch element (clamp semantics fall out of the prefix rows)
  - logits via fused tensor_tensor_reduce over [c|s]-interleaved pairs
  - attention-weighted sums via scalar_tensor_tensor accumulation chains
"""

import sys

if "/opt/trn_rl_repo" not in sys.path:
    sys.path.insert(0, "/opt/trn_rl_repo")

import numpy as np
import ml_dtypes

import concourse.bass as bass
import concourse.bacc as bacc
import concourse.tile as tile
from concourse import mybir
from concourse.bass_utils import run_bass_kernel_spmd

N_CORES = 8
B = 16384
BL = B // N_CORES          # 2048 per core
P = 128
T = BL // P                # 16 tiles per core
D = 512
W = 5
N_ENTITY = 100000
N_RELATION = 256
N_DAYROWS = 367            # time_table rows
PAD_DAY = 365              # negatives clamp to this row

F32 = mybir.dt.float32
BF16 = mybir.dt.bfloat16
I32 = mybir.dt.int32

AF = mybir.ActivationFunctionType
OP = mybir.AluOpType


from concourse._compat import with_exitstack


@with_exitstack
def _emit(ctx, tc, outs, ins):
    """Emit the per-core program. outs/ins are dicts of DRAM APs."""
    nc = tc.nc
    embE2 = ins["embE2"]      # [2*N_ENTITY, D] f32   (real rows then img rows)
    embR2 = ins["embR2"]      # [2*N_RELATION, D] f32
    cs_ext = ins["cs_ext"]    # [370, 2*D] bf16 ([cos|sin], 4 prefix rows = row 365)
    ht_idx = ins["ht_idx"]    # [P, T*4] i32  (h, h+NE, t, t+NE per tile col)
    r_idx = ins["r_idx"]      # [P, T*2] i32
    d_idx = ins["d_idx"]      # [P, T]   i32
    out = outs["out"]         # [P, T] f32

    singles = ctx.enter_context(tc.tile_pool(name="singles", bufs=1))
    gpool = ctx.enter_context(tc.tile_pool(name="g", bufs=3))
    upool = ctx.enter_context(tc.tile_pool(name="u", bufs=2))
    wpool = ctx.enter_context(tc.tile_pool(name="w", bufs=2))
    spool = ctx.enter_context(tc.tile_pool(name="s", bufs=2))

    # --- load index tiles + output accumulator (resident) ---
    sb_ht = singles.tile([P, T * 4], I32)
    sb_r = singles.tile([P, T * 2], I32)
    sb_d = singles.tile([P, T], I32)
    out_acc = singles.tile([P, T], F32)
    nc.sync.dma_start(sb_ht[:], ht_idx[:])
    nc.sync.dma_start(sb_r[:], r_idx[:])
    nc.sync.dma_start(sb_d[:], d_idx[:])

    for t in range(T):
        # ---- gathers ----
        g = gpool.tile([P, 4 * D], BF16, tag="g")      # hr|hi|tr|ti
        rg = gpool.tile([P, 2 * D], BF16, tag="rg")    # rr|ri
        cs = gpool.tile([P, W * 2 * D], BF16, tag="cs")  # per w: [c_w | s_w]

        for c in range(4):
            nc.gpsimd.indirect_dma_start(
                out=g[:, c * D:(c + 1) * D],
                out_offset=None,
                in_=embE2[:],
                in_offset=bass.IndirectOffsetOnAxis(
                    ap=sb_ht[:, t * 4 + c: t * 4 + c + 1], axis=0
                ),
            )
        for c in range(2):
            nc.gpsimd.indirect_dma_start(
                out=rg[:, c * D:(c + 1) * D],
                out_offset=None,
                in_=embR2[:],
                in_offset=bass.IndirectOffsetOnAxis(
                    ap=sb_r[:, t * 2 + c: t * 2 + c + 1], axis=0
                ),
            )
        nc.gpsimd.indirect_dma_start(
            out=cs[:],
            out_offset=None,
            in_=cs_ext[:],
            in_offset=bass.IndirectOffsetOnAxis(ap=sb_d[:, t: t + 1], axis=0),
        )

        hr = g[:, 0 * D:1 * D]
        hi = g[:, 1 * D:2 * D]
        tr = g[:, 2 * D:3 * D]
        ti = g[:, 3 * D:4 * D]
        rr = rg[:, 0 * D:1 * D]
        ri = rg[:, 1 * D:2 * D]

        # ---- u-pairs: coefficient of c | coefficient of s for each logit type
        # type 0 (real,h): [ rr*hr | -rr*hi ]
        # type 1 (img ,h): [ ri*hi |  ri*hr ]
        # type 2 (real,t): [ rr*tr | -rr*ti ]
        # type 3 (img ,t): [ ri*ti |  ri*tr ]
        U = upool.tile([P, 4, 2 * D], BF16, tag="U")
        nrr = spool.tile([P, D], BF16, tag="nrr")
        nc.vector.tensor_scalar(out=nrr[:], in0=rr, scalar1=-1.0, scalar2=None,
                                op0=OP.mult)
        nc.vector.tensor_tensor(out=U[:, 0, 0:D], in0=rr, in1=hr, op=OP.mult)
        nc.vector.tensor_tensor(out=U[:, 0, D:2 * D], in0=nrr[:], in1=hi,
                                op=OP.mult)
        nc.vector.tensor_tensor(out=U[:, 1, 0:D], in0=ri, in1=hi, op=OP.mult)
        nc.vector.tensor_tensor(out=U[:, 1, D:2 * D], in0=ri, in1=hr, op=OP.mult)
        nc.vector.tensor_tensor(out=U[:, 2, 0:D], in0=rr, in1=tr, op=OP.mult)
        nc.vector.tensor_tensor(out=U[:, 2, D:2 * D], in0=nrr[:], in1=ti,
                                op=OP.mult)
        nc.vector.tensor_tensor(out=U[:, 3, 0:D], in0=ri, in1=ti, op=OP.mult)
        nc.vector.tensor_tensor(out=U[:, 3, D:2 * D], in0=ri, in1=tr, op=OP.mult)

        # ---- logits: L[b, ty, w] = sum(U[ty] * cs[w]) ----
        # DVE: one broadcast TT per type over all 5 windows;
        # ACT: per-(ty,w) Copy with accum_out does the reduction.
        L = spool.tile([P, 4 * W], F32, tag="L")
        dummy = spool.tile([P, 2 * D], BF16, tag="dummy")
        csv = cs.rearrange("p (w e) -> p w e", w=W)
        # types 0,1: fused STT dot on DVE; types 2,3: DVE broadcast-mult
        # + ACT accum reduce (balances DVE vs ACT)
        for ty in range(2):
            for w in range(W):
                nc.vector.scalar_tensor_tensor(
                    out=dummy[:], in0=U[:, ty, :], scalar=1.0,
                    in1=csv[:, w, :], op0=OP.mult, op1=OP.mult,
                    accum_out=L[:, ty * W + w: ty * W + w + 1])
        for ty in range(2, 4):
            prod5 = wpool.tile([P, W, 2 * D], BF16, tag="prod5")
            ub = U[:, ty: ty + 1, :].to_broadcast([P, W, 2 * D])
            nc.vector.tensor_tensor(out=prod5[:], in0=ub, in1=csv, op=OP.mult)
            for w in range(W):
                nc.scalar.activation(
                    dummy[:], prod5[:, w, :], AF.Copy,
                    accum_out=L[:, ty * W + w: ty * W + w + 1])

        # ---- softmax over w (logits are O(1); skip max-subtraction) ----
        Ex = spool.tile([P, 4 * W], F32, tag="Ex")
        Sm = spool.tile([P, 4], F32, tag="Sm")
        Rc = spool.tile([P, 4], F32, tag="Rc")
        Al = spool.tile([P, 4 * W], F32, tag="Al")
        nc.scalar.activation(Ex[:], L[:], AF.Exp)
        nc.vector.tensor_reduce(
            out=Sm[:], in_=Ex.rearrange("p (t w) -> p t w", w=W),
            axis=mybir.AxisListType.X, op=OP.add)
        nc.vector.reciprocal(Rc[:], Sm[:])
        for ty in range(4):
            nc.vector.tensor_scalar(
                out=Al[:, ty * W:(ty + 1) * W],
                in0=Ex[:, ty * W:(ty + 1) * W],
                scalar1=Rc[:, ty: ty + 1],
                scalar2=None,
                op0=OP.mult,
            )

        # ---- attention-weighted sums: CSS[ty] = sum_w alpha[ty,w]*cs[w] ----
        # ACT: 5 scaled copies (scale = alpha per partition); DVE: tree-add.
        CSS = wpool.tile([P, 4, 2 * D], BF16, tag="CSS")
        for ty in range(4):
            ap5 = wpool.tile([P, W, 2 * D], BF16, tag="ap5")
            for w in range(W):
                # split scaled copies between ACT (3) and DVE-TS (2)
                if w < 3:
                    nc.scalar.activation(
                        ap5[:, w, :], csv[:, w, :], AF.Copy,
                        scale=Al[:, ty * W + w: ty * W + w + 1])
                else:
                    nc.vector.tensor_scalar(
                        out=ap5[:, w, :], in0=csv[:, w, :],
                        scalar1=Al[:, ty * W + w: ty * W + w + 1],
                        scalar2=None, op0=OP.mult)
            t12 = spool.tile([P, 2, 2 * D], BF16, tag="t12")
            # one wide add: (p0+p2 | p1+p3), then fold halves, then +p4
            nc.vector.tensor_tensor(
                out=t12[:], in0=ap5[:, 0:2, :], in1=ap5[:, 2:4, :], op=OP.add)
            eng = nc.vector if ty < 2 else nc.gpsimd
            eng.tensor_tensor(out=t12[:, 0, :], in0=t12[:, 0, :],
                              in1=t12[:, 1, :], op=OP.add)
            eng.tensor_tensor(out=CSS[:, ty, :], in0=t12[:, 0, :],
                              in1=ap5[:, 4, :], op=OP.add)

        # ---- recombine: y/z vectors [P, D] ----
        # y_r = hr*Cc0 - hi*Cs0 ; y_i = hr*Cs1 + hi*Cc1
        # z_r = tr*Cc2 - ti*Cs2 ; z_i = tr*Cs3 + ti*Cc3
        p1 = spool.tile([P, D], BF16, tag="p1")
        p2 = spool.tile([P, D], BF16, tag="p2")
        yz = wpool.tile([P, 4, D], BF16, tag="yz")
        specs = [
            (hr, CSS[:, 0, 0:D], hi, CSS[:, 0, D:2 * D], OP.subtract),  # y_r
            (hr, CSS[:, 1, D:2 * D], hi, CSS[:, 1, 0:D], OP.add),       # y_i
            (tr, CSS[:, 2, 0:D], ti, CSS[:, 2, D:2 * D], OP.subtract),  # z_r
            (tr, CSS[:, 3, D:2 * D], ti, CSS[:, 3, 0:D], OP.add),       # z_i
        ]
        for k, (a0, b0, a1, b1, opk) in enumerate(specs):
            # y_i/z_r/z_i products+combine go to GpSimd to offload DVE
            eng = nc.vector if k < 1 else nc.gpsimd
            pa = p1 if k < 1 else spool.tile([P, D], BF16, tag=f"gp{k}a")
            pb = p2 if k < 1 else spool.tile([P, D], BF16, tag=f"gp{k}b")
            eng.tensor_tensor(out=pa[:], in0=a0, in1=b0, op=OP.mult)
            eng.tensor_tensor(out=pb[:], in0=a1, in1=b1, op=OP.mult)
            eng.tensor_tensor(out=yz[:, k, :], in0=pa[:], in1=pb[:], op=opk)

        # ---- final: out = sum|y_r + rr - z_r| + sum|y_i + ri + z_i| ----
        f1 = spool.tile([P, D], BF16, tag="f1")
        f2 = spool.tile([P, D], BF16, tag="f2")
        o_r = spool.tile([P, 1], F32, tag="o_r")
        o_i = spool.tile([P, 1], F32, tag="o_i")
        nc.vector.tensor_tensor(out=f1[:], in0=yz[:, 0, :], in1=rr, op=OP.add)
        nc.vector.tensor_tensor(out=f2[:], in0=f1[:], in1=yz[:, 2, :], op=OP.subtract)
        nc.vector.tensor_reduce(
            out=o_r[:], in_=f2[:], axis=mybir.AxisListType.X, op=OP.add,
            apply_absolute_value=True)
        nc.vector.tensor_tensor(out=f1[:], in0=yz[:, 1, :], in1=ri, op=OP.add)
        nc.vector.tensor_tensor(out=f2[:], in0=f1[:], in1=yz[:, 3, :], op=OP.add)
        nc.vector.tensor_reduce(
            out=o_i[:], in_=f2[:], axis=mybir.AxisListType.X, op=OP.add,
            apply_absolute_value=True)
        nc.vector.tensor_tensor(
            out=out_acc[:, t: t + 1], in0=o_r[:], in1=o_i[:], op=OP.add)

    nc.sync.dma_start(out[:], out_acc[:])


def _host_prep(h_i, t_i, r_i, d_i, emb_E_real, emb_E_img, emb_R_real,
               emb_R_img, time_table):
    """Host-side layout prep (cheap index/table manipulation only)."""
    embE2 = np.ascontiguousarray(
        np.concatenate([emb_E_real, emb_E_img], axis=0), dtype=np.float32)
    embR2 = np.ascontiguousarray(
        np.concatenate([emb_R_real, emb_R_img], axis=0), dtype=np.float32)
    tt = np.asarray(time_table, dtype=np.float32)
    cs = np.concatenate([np.cos(tt), np.sin(tt)], axis=1)  # [367, 1024] f32
    # 4 prefix rows equal to row PAD_DAY implement the negative-index clamp;
    # row d of the original table sits at ext row d+4, so a window gather for
    # batch element b is rows d[b] .. d[b]+4 of cs_ext == one contiguous read.
    cs_ext = np.concatenate(
        [np.broadcast_to(cs[PAD_DAY], (4, 2 * D)), cs[:366]], axis=0)
    cs_ext = np.ascontiguousarray(cs_ext, dtype=ml_dtypes.bfloat16)  # [370, 1024]

    ht = np.stack(
        [h_i, h_i + N_ENTITY, t_i, t_i + N_ENTITY], axis=1).astype(np.int32)
    rx = np.stack([r_i, r_i + N_RELATION], axis=1).astype(np.int32)
    dx = d_i.astype(np.int32).reshape(B, 1)

    def tileize(a):
        # [BL, C] -> [P, T*C] with element [p, t*C+c] = a[t*P+p, c]
        C = a.shape[1]
        return np.ascontiguousarray(
            a.reshape(T, P, C).transpose(1, 0, 2).reshape(P, T * C))

    in_maps = []
    for core in range(N_CORES):
        sl = slice(core * BL, (core + 1) * BL)
        in_maps.append(dict(
            embE2=embE2,
            embR2=embR2,
            cs_ext=cs_ext,
            ht_idx=tileize(ht[sl]),
            r_idx=tileize(rx[sl]),
            d_idx=tileize(dx[sl]),
        ))
    return in_maps


def build_nc():
    nc = bacc.Bacc(
        "TRN2",
        target_bir_lowering=False,
        debug=False,
        enable_asserts=False,
        num_devices=N_CORES,
    )
    ins = dict(
        embE2=nc.dram_tensor("embE2", [2 * N_ENTITY, D], F32,
                             kind="ExternalInput").ap(),
        embR2=nc.dram_tensor("embR2", [2 * N_RELATION, D], F32,
                             kind="ExternalInput").ap(),
        cs_ext=nc.dram_tensor("cs_ext", [370, 2 * D], BF16,
                              kind="ExternalInput").ap(),
        ht_idx=nc.dram_tensor("ht_idx", [P, T * 4], I32,
                              kind="ExternalInput").ap(),
        r_idx=nc.dram_tensor("r_idx", [P, T * 2], I32,
                             kind="ExternalInput").ap(),
        d_idx=nc.dram_tensor("d_idx", [P, T], I32,
                             kind="ExternalInput").ap(),
    )
    outs = dict(
        out=nc.dram_tensor("out", [P, T], F32, kind="ExternalOutput").ap(),
    )
    with tile.TileContext(nc) as tc:
        _emit(tc, outs, ins)
    nc.compile()
    return nc


_NC_CACHE = {}


def kernel(h_i, t_i, r_i, d_i, emb_E_real, emb_E_img, emb_R_real, emb_R_img,
           time_table, _want_results=False, _trace=False):
    in_maps = _host_prep(h_i, t_i, r_i, d_i, emb_E_real, emb_E_img,
                         emb_R_real, emb_R_img, time_table)
    if "nc" not in _NC_CACHE:
        _NC_CACHE["nc"] = build_nc()
    nc = _NC_CACHE["nc"]
    res = run_bass_kernel_spmd(
        nc, in_maps, core_ids=list(range(N_CORES)), trace=_trace)
    out = np.empty((B,), np.float32)
    for core in range(N_CORES):
        o = res.results[core]["out"]  # [P, T]
        out[core * BL:(core + 1) * BL] = np.asarray(o).T.reshape(BL)
    if _want_results:
        return out, res
    return out


# revision 13
# speedup vs baseline: 1.1079x; 1.1079x over previous
"""ContxE-style temporal KG embedding scoring kernel for Trainium2 (Bass/Tile).

Contract: kernel(**inputs) takes FULL unsharded numpy inputs and returns the
FULL [B] float32 output. Internally shards the batch across 8 NeuronCores
(data-parallel, tables replicated) and runs a Bass/Tile kernel via
run_bass_kernel_spmd.

Math (per batch element b, window W=5, D=512):
  idx[b,w] = d[b]-(4-w), clamped: negatives -> 365
  c/s[b,w,:] = cos/sin(time_table[idx[b,w]])
  h_real = hr*c - hi*s ; h_img = hr*s + hi*c   (same for t)
  4 attention softmaxes over W of <r, rotated>, then weighted sums,
  out = sum|y_r + rr - z_r| + sum|y_i + ri + z_i|

Device-side strategy (per core, B_loc=2048 = 16 tiles of 128):
  - ONE indirect-DMA gather per embedding pair (tables concatenated host-side)
  - cos|sin rows come from a host-precomputed bf16 table with 4 prefix rows
    equal to row 365, so the W-window gather is ONE contiguous 10KB read per
    batch element (clamp semantics fall out of the prefix rows)
  - logits via fused tensor_tensor_reduce over [c|s]-interleaved pairs
  - attention-weighted sums via scalar_tensor_tensor accumulation chains
"""

import sys

if "/opt/trn_rl_repo" not in sys.path:
    sys.path.insert(0, "/opt/trn_rl_repo")

import numpy as np
import ml_dtypes

import concourse.bass as bass
import concourse.bacc as bacc
import concourse.tile as tile
from concourse import mybir
from concourse.bass_utils import run_bass_kernel_spmd

N_CORES = 8
B = 16384
BL = B // N_CORES          # 2048 per core
P = 128
T = BL // P                # 16 tiles per core
D = 512
W = 5
N_ENTITY = 100000
N_RELATION = 256
N_DAYROWS = 367            # time_table rows
PAD_DAY = 365              # negatives clamp to this row

F32 = mybir.dt.float32
BF16 = mybir.dt.bfloat16
I32 = mybir.dt.int32

AF = mybir.ActivationFunctionType
OP = mybir.AluOpType


from concourse._compat import with_exitstack


@with_exitstack
def _emit(ctx, tc, outs, ins):
    """Emit the per-core program. outs/ins are dicts of DRAM APs."""
    nc = tc.nc
    embE2 = ins["embE2"]      # [2*N_ENTITY, D] f32   (real rows then img rows)
    embR2 = ins["embR2"]      # [2*N_RELATION, D] f32
    cs_ext = ins["cs_ext"]    # [370, 2*D] bf16 ([cos|sin], 4 prefix rows = row 365)
    ht_idx = ins["ht_idx"]    # [P, T*4] i32  (h, h+NE, t, t+NE per tile col)
    r_idx = ins["r_idx"]      # [P, T*2] i32
    d_idx = ins["d_idx"]      # [P, T]   i32
    out = outs["out"]         # [P, T] f32

    singles = ctx.enter_context(tc.tile_pool(name="singles", bufs=1))
    gpool = ctx.enter_context(tc.tile_pool(name="g", bufs=3))
    upool = ctx.enter_context(tc.tile_pool(name="u", bufs=2))
    wpool = ctx.enter_context(tc.tile_pool(name="w", bufs=2))
    spool = ctx.enter_context(tc.tile_pool(name="s", bufs=2))

    # --- load index tiles + output accumulator (resident) ---
    sb_ht = singles.tile([P, T * 4], I32)
    sb_r = singles.tile([P, T * 2], I32)
    sb_d = singles.tile([P, T], I32)
    out_acc = singles.tile([P, T], F32)
    nc.sync.dma_start(sb_ht[:], ht_idx[:])
    nc.sync.dma_start(sb_r[:], r_idx[:])
    nc.sync.dma_start(sb_d[:], d_idx[:])

    for t in range(T):
        # ---- gathers ----
        g = gpool.tile([P, 4 * D], BF16, tag="g")      # hr|hi|tr|ti
        rg = gpool.tile([P, 2 * D], BF16, tag="rg")    # rr|ri
        cs = gpool.tile([P, W * 2 * D], BF16, tag="cs")  # per w: [c_w | s_w]

        for c in range(4):
            nc.gpsimd.indirect_dma_start(
                out=g[:, c * D:(c + 1) * D],
                out_offset=None,
                in_=embE2[:],
                in_offset=bass.IndirectOffsetOnAxis(
                    ap=sb_ht[:, t * 4 + c: t * 4 + c + 1], axis=0
                ),
            )
        for c in range(2):
            nc.gpsimd.indirect_dma_start(
                out=rg[:, c * D:(c + 1) * D],
                out_offset=None,
                in_=embR2[:],
                in_offset=bass.IndirectOffsetOnAxis(
                    ap=sb_r[:, t * 2 + c: t * 2 + c + 1], axis=0
                ),
            )
        nc.gpsimd.indirect_dma_start(
            out=cs[:],
            out_offset=None,
            in_=cs_ext[:],
            in_offset=bass.IndirectOffsetOnAxis(ap=sb_d[:, t: t + 1], axis=0),
        )

        hr = g[:, 0 * D:1 * D]
        hi = g[:, 1 * D:2 * D]
        tr = g[:, 2 * D:3 * D]
        ti = g[:, 3 * D:4 * D]
        rr = rg[:, 0 * D:1 * D]
        ri = rg[:, 1 * D:2 * D]

        # ---- u-pairs: coefficient of c | coefficient of s for each logit type
        # type 0 (real,h): [ rr*hr | -rr*hi ]
        # type 1 (img ,h): [ ri*hi |  ri*hr ]
        # type 2 (real,t): [ rr*tr | -rr*ti ]
        # type 3 (img ,t): [ ri*ti |  ri*tr ]
        U = upool.tile([P, 4, 2 * D], BF16, tag="U")
        nrr = spool.tile([P, D], BF16, tag="nrr")
        nc.vector.tensor_scalar(out=nrr[:], in0=rr, scalar1=-1.0, scalar2=None,
                                op0=OP.mult)
        nc.vector.tensor_tensor(out=U[:, 0, 0:D], in0=rr, in1=hr, op=OP.mult)
        nc.vector.tensor_tensor(out=U[:, 0, D:2 * D], in0=nrr[:], in1=hi,
                                op=OP.mult)
        nc.vector.tensor_tensor(out=U[:, 1, 0:D], in0=ri, in1=hi, op=OP.mult)
        nc.vector.tensor_tensor(out=U[:, 1, D:2 * D], in0=ri, in1=hr, op=OP.mult)
        nc.vector.tensor_tensor(out=U[:, 2, 0:D], in0=rr, in1=tr, op=OP.mult)
        nc.vector.tensor_tensor(out=U[:, 2, D:2 * D], in0=nrr[:], in1=ti,
                                op=OP.mult)
        nc.vector.tensor_tensor(out=U[:, 3, 0:D], in0=ri, in1=ti, op=OP.mult)
        nc.vector.tensor_tensor(out=U[:, 3, D:2 * D], in0=ri, in1=tr, op=OP.mult)

        # ---- logits: L[b, ty, w] = sum(U[ty] * cs[w]) ----
        # DVE: one broadcast TT per type over all 5 windows;
        # ACT: per-(ty,w) Copy with accum_out does the reduction.
        L = spool.tile([P, 4 * W], F32, tag="L")
        dummy = spool.tile([P, 2 * D], BF16, tag="dummy")
        csv = cs.rearrange("p (w e) -> p w e", w=W)
        # types 0,1: fused STT dot on DVE; types 2,3: DVE broadcast-mult
        # + ACT accum reduce (balances DVE vs ACT)
        for ty in range(2):
            for w in range(W):
                nc.vector.scalar_tensor_tensor(
                    out=dummy[:], in0=U[:, ty, :], scalar=1.0,
                    in1=csv[:, w, :], op0=OP.mult, op1=OP.mult,
                    accum_out=L[:, ty * W + w: ty * W + w + 1])
        for ty in range(2, 4):
            prod5 = wpool.tile([P, W, 2 * D], BF16, tag="prod5")
            ub = U[:, ty: ty + 1, :].to_broadcast([P, W, 2 * D])
            nc.vector.tensor_tensor(out=prod5[:], in0=ub, in1=csv, op=OP.mult)
            for w in range(W):
                nc.scalar.activation(
                    dummy[:], prod5[:, w, :], AF.Copy,
                    accum_out=L[:, ty * W + w: ty * W + w + 1])

        # ---- softmax over w (logits are O(1); skip max-subtraction) ----
        Ex = spool.tile([P, 4 * W], F32, tag="Ex")
        Sm = spool.tile([P, 4], F32, tag="Sm")
        Rc = spool.tile([P, 4], F32, tag="Rc")
        Al = spool.tile([P, 4 * W], F32, tag="Al")
        nc.scalar.activation(Ex[:], L[:], AF.Exp)
        nc.vector.tensor_reduce(
            out=Sm[:], in_=Ex.rearrange("p (t w) -> p t w", w=W),
            axis=mybir.AxisListType.X, op=OP.add)
        nc.vector.reciprocal(Rc[:], Sm[:])
        for ty in range(4):
            nc.vector.tensor_scalar(
                out=Al[:, ty * W:(ty + 1) * W],
                in0=Ex[:, ty * W:(ty + 1) * W],
                scalar1=Rc[:, ty: ty + 1],
                scalar2=None,
                op0=OP.mult,
            )

        # ---- attention-weighted sums: CSS[ty] = sum_w alpha[ty,w]*cs[w] ----
        # ACT: 5 scaled copies (scale = alpha per partition); DVE: tree-add.
        CSS = wpool.tile([P, 4, 2 * D], BF16, tag="CSS")
        for ty in range(4):
            ap5 = wpool.tile([P, W, 2 * D], BF16, tag="ap5")
            for w in range(W):
                # split scaled copies between ACT (3) and DVE-TS (2)
                if w < 3:
                    nc.scalar.activation(
                        ap5[:, w, :], csv[:, w, :], AF.Copy,
                        scale=Al[:, ty * W + w: ty * W + w + 1])
                else:
                    nc.vector.tensor_scalar(
                        out=ap5[:, w, :], in0=csv[:, w, :],
                        scalar1=Al[:, ty * W + w: ty * W + w + 1],
                        scalar2=None, op0=OP.mult)
            t12 = spool.tile([P, 2, 2 * D], BF16, tag="t12")
            # one wide add: (p0+p2 | p1+p3), then fold halves, then +p4
            nc.vector.tensor_tensor(
                out=t12[:], in0=ap5[:, 0:2, :], in1=ap5[:, 2:4, :], op=OP.add)
            nc.vector.tensor_tensor(out=t12[:, 0, :], in0=t12[:, 0, :],
                                    in1=t12[:, 1, :], op=OP.add)
            nc.vector.tensor_tensor(out=CSS[:, ty, :], in0=t12[:, 0, :],
                                    in1=ap5[:, 4, :], op=OP.add)

        # ---- recombine: y/z vectors [P, D] ----
        # y_r = hr*Cc0 - hi*Cs0 ; y_i = hr*Cs1 + hi*Cc1
        # z_r = tr*Cc2 - ti*Cs2 ; z_i = tr*Cs3 + ti*Cc3
        p1 = spool.tile([P, D], BF16, tag="p1")
        p2 = spool.tile([P, D], BF16, tag="p2")
        yz = wpool.tile([P, 4, D], BF16, tag="yz")
        specs = [
            (hr, CSS[:, 0, 0:D], hi, CSS[:, 0, D:2 * D], OP.subtract),  # y_r
            (hr, CSS[:, 1, D:2 * D], hi, CSS[:, 1, 0:D], OP.add),       # y_i
            (tr, CSS[:, 2, 0:D], ti, CSS[:, 2, D:2 * D], OP.subtract),  # z_r
            (tr, CSS[:, 3, D:2 * D], ti, CSS[:, 3, 0:D], OP.add),       # z_i
        ]
        for k, (a0, b0, a1, b1, opk) in enumerate(specs):
            # y_i/z_r/z_i products+combine go to GpSimd to offload DVE
            eng = nc.vector if k < 1 else nc.gpsimd
            pa = p1 if k < 1 else spool.tile([P, D], BF16, tag=f"gp{k}a")
            pb = p2 if k < 1 else spool.tile([P, D], BF16, tag=f"gp{k}b")
            eng.tensor_tensor(out=pa[:], in0=a0, in1=b0, op=OP.mult)
            eng.tensor_tensor(out=pb[:], in0=a1, in1=b1, op=OP.mult)
            eng.tensor_tensor(out=yz[:, k, :], in0=pa[:], in1=pb[:], op=opk)

        # ---- final: out = sum|y_r + rr - z_r| + sum|y_i + ri + z_i| ----
        f1 = spool.tile([P, D], BF16, tag="f1")
        f2 = spool.tile([P, D], BF16, tag="f2")
        o_r = spool.tile([P, 1], F32, tag="o_r")
        o_i = spool.tile([P, 1], F32, tag="o_i")
        nc.vector.tensor_tensor(out=f1[:], in0=yz[:, 0, :], in1=rr, op=OP.add)
        nc.vector.tensor_tensor(out=f2[:], in0=f1[:], in1=yz[:, 2, :], op=OP.subtract)
        nc.vector.tensor_reduce(
            out=o_r[:], in_=f2[:], axis=mybir.AxisListType.X, op=OP.add,
            apply_absolute_value=True)
        nc.vector.tensor_tensor(out=f1[:], in0=yz[:, 1, :], in1=ri, op=OP.add)
        nc.vector.tensor_tensor(out=f2[:], in0=f1[:], in1=yz[:, 3, :], op=OP.add)
        nc.vector.tensor_reduce(
            out=o_i[:], in_=f2[:], axis=mybir.AxisListType.X, op=OP.add,
            apply_absolute_value=True)
        nc.vector.tensor_tensor(
            out=out_acc[:, t: t + 1], in0=o_r[:], in1=o_i[:], op=OP.add)

    nc.sync.dma_start(out[:], out_acc[:])


def _host_prep(h_i, t_i, r_i, d_i, emb_E_real, emb_E_img, emb_R_real,
               emb_R_img, time_table):
    """Host-side layout prep (cheap index/table manipulation only)."""
    embE2 = np.ascontiguousarray(
        np.concatenate([emb_E_real, emb_E_img], axis=0), dtype=np.float32)
    embR2 = np.ascontiguousarray(
        np.concatenate([emb_R_real, emb_R_img], axis=0), dtype=np.float32)
    tt = np.asarray(time_table, dtype=np.float32)
    cs = np.concatenate([np.cos(tt), np.sin(tt)], axis=1)  # [367, 1024] f32
    # 4 prefix rows equal to row PAD_DAY implement the negative-index clamp;
    # row d of the original table sits at ext row d+4, so a window gather for
    # batch element b is rows d[b] .. d[b]+4 of cs_ext == one contiguous read.
    cs_ext = np.concatenate(
        [np.broadcast_to(cs[PAD_DAY], (4, 2 * D)), cs[:366]], axis=0)
    cs_ext = np.ascontiguousarray(cs_ext, dtype=ml_dtypes.bfloat16)  # [370, 1024]

    ht = np.stack(
        [h_i, h_i + N_ENTITY, t_i, t_i + N_ENTITY], axis=1).astype(np.int32)
    rx = np.stack([r_i, r_i + N_RELATION], axis=1).astype(np.int32)
    dx = d_i.astype(np.int32).reshape(B, 1)

    def tileize(a):
        # [BL, C] -> [P, T*C] with element [p, t*C+c] = a[t*P+p, c]
        C = a.shape[1]
        return np.ascontiguousarray(
            a.reshape(T, P, C).transpose(1, 0, 2).reshape(P, T * C))

    in_maps = []
    for core in range(N_CORES):
        sl = slice(core * BL, (core + 1) * BL)
        in_maps.append(dict(
            embE2=embE2,
            embR2=embR2,
            cs_ext=cs_ext,
            ht_idx=tileize(ht[sl]),
            r_idx=tileize(rx[sl]),
            d_idx=tileize(dx[sl]),
        ))
    return in_maps


def build_nc():
    nc = bacc.Bacc(
        "TRN2",
        target_bir_lowering=False,
        debug=False,
        enable_asserts=False,
        num_devices=N_CORES,
    )
    ins = dict(
        embE2=nc.dram_tensor("embE2", [2 * N_ENTITY, D], F32,
                             kind="ExternalInput").ap(),
        embR2=nc.dram_tensor("embR2", [2 * N_RELATION, D], F32,
                             kind="ExternalInput").ap(),
        cs_ext=nc.dram_tensor("cs_ext", [370, 2 * D], BF16,
                              kind="ExternalInput").ap(),
        ht_idx=nc.dram_tensor("ht_idx", [P, T * 4], I32,
                              kind="ExternalInput").ap(),
        r_idx=nc.dram_tensor("r_idx", [P, T * 2], I32,
                             kind="ExternalInput").ap(),
        d_idx=nc.dram_tensor("d_idx", [P, T], I32,
                             kind="ExternalInput").ap(),
    )
    outs = dict(
        out=nc.dram_tensor("out", [P, T], F32, kind="ExternalOutput").ap(),
    )
    with tile.TileContext(nc) as tc:
        _emit(tc, outs, ins)
    nc.compile()
    return nc


_NC_CACHE = {}


def kernel(h_i, t_i, r_i, d_i, emb_E_real, emb_E_img, emb_R_real, emb_R_img,
           time_table, _want_results=False, _trace=False):
    in_maps = _host_prep(h_i, t_i, r_i, d_i, emb_E_real, emb_E_img,
                         emb_R_real, emb_R_img, time_table)
    if "nc" not in _NC_CACHE:
        _NC_CACHE["nc"] = build_nc()
    nc = _NC_CACHE["nc"]
    res = run_bass_kernel_spmd(
        nc, in_maps, core_ids=list(range(N_CORES)), trace=_trace)
    out = np.empty((B,), np.float32)
    for core in range(N_CORES):
        o = res.results[core]["out"]  # [P, T]
        out[core * BL:(core + 1) * BL] = np.asarray(o).T.reshape(BL)
    if _want_results:
        return out, res
    return out
